# revision 1
# baseline (speedup 1.0000x reference)
"""Trainium2 Bass kernel for causal top-K (K=8) similarity message passing.

Math per batch b (reference):
  gate  = sigmoid(x @ w_gate + b_gate)                      (B,T)
  S     = x @ x^T, causal-masked to NEG=-1e30               (B,T,T)
  top-8 per row -> selected neighbour set, count=min(i+1,8)
  msg   = mean of selected x rows
  blend = mix*x + (1-mix)*msg
  out   = gate * gelu(blend*gain + bias) * (softplus(log_scale)+0.01)

Sharding: 8 cores = 4 batches x 2 query-parity shards. Core c handles
batch b=c>>1, parity p=c&1, processing query tiles g = 2t+p (t=0..T/256-1)
of 128 rows each. One uniform program for all cores; all parity
dependence is carried in per-core input data (masks / select scalars).

Per-core input is ONLY half of x[b] in fp16 (4MB, zero-copy upload;
the pair AllGathers the full batch on device) plus one packed ~0.5MB
param/mask tensor. Everything else is computed on device:

  xfull   = pair AllGather of the two contiguous x[b] halves
  xT      = xfull^T built once via PE transposes (fp16)
  per query tile t (Lc = 2t+2 key chunks of 128):
    xqT    = c1*xT[:, 2t*128 win] + c2*xT[:, (2t+1)*128 win]
             (both parity candidates live in xT; c1/c2 are input data)
    scores = xqT^T @ xT (PE, fp16 in / f32 acc) -> SBUF + causal masks
    v8     = max8(scores), tau = v8[:,7]       (DVE top-8 instruction)
    sel    = scores >= tau (0/1), fixed up for tile 0, diagonal gets
             mix*count/(1-mix) added
    pm     = sum_c sel_chunk^T-transposed @ x_chunk        (PSUM f32)
    msg    = pm * (1-mix)/count          (per-row scale = blend[q, d])
    y      = gate * transpose(Gelu(transpose-slab(msg)*gainT + biasT))
             (gain/bias/gelu applied in feature-on-partition layout)

Output y is fp16 (halves the download); the host deinterleaves and
upcasts via one fused XLA-on-CPU op.
"""

import os
import sys

for _p in ("/opt/trn_rl_repo", os.path.expanduser("~/.axon_site/_ro/trn_rl_repo")):
    if os.path.isdir(_p) and _p not in sys.path:
        sys.path.insert(0, _p)
        break

import numpy as np

import concourse.bacc as bacc
import concourse.mybir as mybir
from concourse import masks
from concourse.tile import TileContext

F32 = mybir.dt.float32
F32R = mybir.dt.float32r
FP16 = mybir.dt.float16
I32 = mybir.dt.int32
AF = mybir.ActivationFunctionType
ALU = mybir.AluOpType
NEG = np.float32(-1e30)

D = 1024
DC = 8  # D // 128
N_CORES = 8

_prog_cache = {}
_runner_cache = {}


def build_program(T, reps=1, sdt=FP16, stage=5):
    """Build + compile the uniform per-core program for sequence length T."""
    key = (T, reps, sdt, stage)
    if key in _prog_cache:
        return _prog_cache[key]

    ODT = FP16 if sdt == FP16 else F32   # output dtype
    PDT = sdt                            # PSUM dtype for sdt transposes

    NQT = T // 256  # query tiles per core
    nc = bacc.Bacc(trn_type="TRN2", target_bir_lowering=False, debug=False,
                   num_devices=N_CORES, dynamic_dma_scratch_size=512)

    NP = 4 * 256 + NQT + 2 * DC + 4      # packed f32 param columns
    x_in = nc.dram_tensor("x", [T // 2, D], sdt, kind="ExternalInput").ap()
    xl = nc.dram_tensor("xl", [T // 2, D], sdt).ap()
    xfull = nc.dram_tensor("xfull", [T, D], sdt).ap()
    prm_in = nc.dram_tensor("prm", [128, NP], F32, kind="ExternalInput").ap()
    WDT = sdt if sdt == FP16 else F32    # gate weight dtype
    wg_in = nc.dram_tensor("wg", [128, DC], WDT, kind="ExternalInput").ap()
    y_out = nc.dram_tensor("y", [NQT, 128, D], ODT, kind="ExternalOutput").ap()

    from contextlib import ExitStack

    with TileContext(nc) as tc, ExitStack() as ctx:
        cpool = ctx.enter_context(tc.tile_pool(name="consts", bufs=1))
        xTp = ctx.enter_context(tc.tile_pool(name="xTp", bufs=1))
        xallp = ctx.enter_context(tc.tile_pool(name="xallp", bufs=1))
        Sp = ctx.enter_context(tc.tile_pool(name="Sp", bufs=2))
        xqp = ctx.enter_context(tc.tile_pool(name="xqp", bufs=2))
        stp = ctx.enter_context(tc.tile_pool(name="stp", bufs=3))
        msgp = ctx.enter_context(tc.tile_pool(name="msgp", bufs=3))
        smallp = ctx.enter_context(tc.tile_pool(name="smallp", bufs=2))
        ps_s = ctx.enter_context(tc.tile_pool(name="ps_s", bufs=2, space="PSUM"))
        ps_t = ctx.enter_context(tc.tile_pool(name="ps_t", bufs=3, space="PSUM"))
        ps_m = ctx.enter_context(tc.tile_pool(name="ps_m", bufs=1, space="PSUM"))
        ps_g = ctx.enter_context(tc.tile_pool(name="ps_g", bufs=1, space="PSUM"))

        prm = cpool.tile([128, NP], F32)
        nc.sync.dma_start(out=prm[:], in_=prm_in[:])
        qmask = prm[:, 0:256]
        smask = prm[:, 256:512]
        dmask = prm[:, 512:768]
        dmask0 = prm[:, 768:1024]
        recip = prm[:, 1024:1024 + NQT]
        gainT = prm[:, 1024 + NQT:1024 + NQT + DC]
        biasT = prm[:, 1024 + NQT + DC:1024 + NQT + 2 * DC]
        sc = prm[:, 1024 + NQT + 2 * DC:1024 + NQT + 2 * DC + 4]
        wg = cpool.tile([128, DC], WDT)
        nc.sync.dma_start(out=wg[:], in_=wg_in[:])
        ident32 = cpool.tile([128, 128], F32)
        masks.make_identity(nc, ident32[:])
        identR = cpool.tile([128, 128], sdt)
        nc.scalar.copy(identR[:], ident32[:])

        groups = [[2 * b, 2 * b + 1] for b in range(N_CORES // 2)]
        for _rep in range(reps):
            # ---- pair AllGather: each core holds half of x[b]; gather the
            # full batch in global row order (even core: rows < T/2) ----
            nc.sync.dma_start(out=xl[:], in_=x_in[:])
            nc.gpsimd.collective_compute(
                "AllGather", mybir.AluOpType.bypass, groups,
                [xl[:]], [xfull[:]])

            # ---- load all key chunks into SBUF once; build
            # xT[dp, dc, k] = x[k, dc*128+dp] via PE transposes ----
            xT = xTp.tile([128, DC, T], sdt)
            xall = xallp.tile([128, T // 128, D], sdt)
            for c in range(T // 128):
                nc.sync.dma_start(out=xall[:, c],
                                  in_=xfull[c * 128:(c + 1) * 128, :])
                for dc in range(DC):
                    pt = ps_t.tile([128, 128], PDT, tag="pt", name="ptr")
                    nc.tensor.transpose(pt[:],
                                        xall[:, c, dc * 128:(dc + 1) * 128],
                                        identR[:])
                    nc.scalar.copy(xT[:, dc, c * 128:(c + 1) * 128], pt[:])

            if stage <= 1:
                dbg = msgp.tile([128, D], ODT, name="dbg")
                nc.vector.tensor_copy(dbg[:], xT[:, 0, 0:D].bitcast(F32))
                nc.sync.dma_start(out=y_out[0], in_=dbg[:])
                continue

            for t in range(NQT):
                Lc = 2 * t + 2
                Lk = Lc * 128

                # ---- extract this core's query tile (parity is data) ----
                # both parity candidates already live transposed inside xT:
                #   xqT = c1*xT[:, :, 2t*128 window] + c2*xT[:, :, (2t+1)*128]
                #       = (w0 - w1)*c1 + w1          (c1, c2=1-c1 in {0,1})
                xqT = xqp.tile([128, DC, 128], sdt, tag="xqT", name="xqT")
                w0 = xT[:, :, 2 * t * 128:(2 * t) * 128 + 128]
                w1 = xT[:, :, (2 * t + 1) * 128:(2 * t + 1) * 128 + 128]
                nc.vector.tensor_sub(xqT[:], w0, w1)
                nc.vector.tensor_scalar(xqT[:], xqT[:], sc[:, 2:3], None,
                                        op0=ALU.mult)
                nc.vector.tensor_add(xqT[:], xqT[:], w1)

                # ---- scores ----
                S = Sp.tile([128, T], F32)
                nblk = (Lk + 511) // 512
                for blk in range(nblk):
                    w = min(512, Lk - blk * 512)
                    ps = ps_s.tile([128, 512], F32)
                    for dc in range(DC):
                        nc.tensor.matmul(ps[:, :w], xqT[:, dc],
                                         xT[:, dc, blk * 512:blk * 512 + w],
                                         start=(dc == 0), stop=(dc == DC - 1))
                    lo = blk * 512
                    plain_w = w if blk < nblk - 1 else w - 256
                    if plain_w > 0:
                        nc.scalar.copy(S[:, lo:lo + plain_w], ps[:, :plain_w])
                    if blk == nblk - 1:
                        nc.vector.tensor_add(S[:, Lk - 256:Lk],
                                             ps[:, w - 256:w], qmask)

                if stage <= 2:
                    dbg2 = msgp.tile([128, D], ODT, name="dbg2")
                    nc.vector.tensor_copy(dbg2[:], S[:, 0:D])
                    nc.sync.dma_start(out=y_out[t], in_=dbg2[:])
                    continue

                # ---- top-8 threshold -> selection weights in-place ----
                v8 = smallp.tile([128, 8], F32, tag="v8", name="v8")
                nc.vector.max(out=v8[:], in_=S[:, :Lk])
                nc.vector.tensor_scalar(S[:, :Lk], S[:, :Lk], v8[:, 7:8], None,
                                        op0=ALU.is_ge)
                if t == 0:
                    nc.vector.tensor_mul(S[:, :256], S[:, :256], smask)
                dm = dmask0 if t == 0 else dmask
                nc.vector.tensor_add(S[:, Lk - 256:Lk], S[:, Lk - 256:Lk], dm)

                if stage <= 3:
                    dbg3 = msgp.tile([128, D], ODT, name="dbg3")
                    nc.vector.tensor_copy(dbg3[:], S[:, 0:D])
                    nc.sync.dma_start(out=y_out[t], in_=dbg3[:])
                    continue

                # ---- gate ----
                pg = ps_g.tile([128, 1], F32)
                for dc in range(DC):
                    lq = (xqT[:, dc] if sdt == FP16
                          else xqT[:, dc].bitcast(F32))
                    nc.tensor.matmul(pg[:], lq, wg[:, dc:dc + 1],
                                     start=(dc == 0), stop=(dc == DC - 1))
                gate = smallp.tile([128, 1], F32, tag="gate", name="gate")
                nc.scalar.activation(gate[:], pg[:], AF.Sigmoid,
                                     bias=sc[:, 0:1], scale=1.0)
                nc.vector.tensor_mul(gate[:], gate[:], sc[:, 1:2])

                # ---- aggregation: pm[q, d] = sum_j selw[q, j] * x[j, d] ----
                pm = ps_m.tile([128, D], F32)
                for c in range(Lc):
                    pt = ps_t.tile([128, 128], F32, tag="pt", name="pts")
                    nc.tensor.transpose(pt[:], S[:, c * 128:(c + 1) * 128],
                                        ident32[:])
                    sT = stp.tile([128, 128], sdt)
                    nc.scalar.copy(sT[:], pt[:])
                    for h in (0, 1):
                        nc.tensor.matmul(pm[:, h * 512:(h + 1) * 512], sT[:],
                                         xall[:, c, h * 512:(h + 1) * 512],
                                         start=(c == 0), stop=(c == Lc - 1))

                # msg = blend[q, d] = pm * (1-mix)/count   (per-row scale)
                msg = msgp.tile([128, D], F32)
                nc.scalar.activation(msg[:], pm[:], AF.Copy,
                                     scale=recip[:, t:t + 1])

                if stage <= 4:
                    dbg4 = msgp.tile([128, D], ODT, name="dbg4")
                    nc.vector.tensor_copy(dbg4[:], msg[:])
                    nc.sync.dma_start(out=y_out[t], in_=dbg4[:])
                    continue

                # ---- tail: y = gate * Gelu(blend^T*gainT + biasT)^T ----
                y = msgp.tile([128, D], ODT, tag="y", name="y")
                for dc in range(DC):
                    pt1 = ps_t.tile([128, 128], F32, tag="pt", name="pt1")
                    nc.tensor.transpose(pt1[:], msg[:, dc * 128:(dc + 1) * 128],
                                        ident32[:])
                    gels = stp.tile([128, 128], F32, tag="gels", name="gels")
                    nc.scalar.activation(gels[:], pt1[:], AF.Gelu,
                                         bias=biasT[:, dc:dc + 1],
                                         scale=gainT[:, dc:dc + 1])
                    pt2 = ps_t.tile([128, 128], F32, tag="pt", name="pt2")
                    nc.tensor.transpose(pt2[:], gels[:], ident32[:])
                    nc.scalar.activation(y[:, dc * 128:(dc + 1) * 128], pt2[:],
                                         AF.Copy, scale=gate[:, 0:1])
                nc.sync.dma_start(out=y_out[t], in_=y[:])

    nc.compile()
    _prog_cache[key] = nc
    return nc


def host_small(p, mix, scale, b_gate, w_gate, gain, bias, T, sdt_np=np.float16):
    """Small per-core input arrays for parity p (everything except x)."""
    NQT = T // 256
    f32 = np.float32

    r = np.arange(128)
    tri_add = np.where(r[None, :] <= r[:, None], f32(0), NEG).astype(f32)
    tri01 = (r[None, :] <= r[:, None]).astype(f32)
    qmask = np.zeros((128, 256), f32)
    smask = np.zeros((128, 256), f32)
    if p == 0:
        qmask[:, :128] = tri_add
        qmask[:, 128:] = NEG
        smask[:, :128] = tri01
    else:
        qmask[:, 128:] = tri_add
        smask[:, :128] = 1.0
        smask[:, 128:] = tri01

    # counts: count(t, q) = min((2t+p)*128 + q + 1, 8)
    g_row = (2 * np.arange(NQT)[:, None] + p) * 128 + r[None, :]  # (NQT,128)
    counts = np.minimum(g_row + 1, 8).astype(f32)

    dmask = np.zeros((128, 256), f32)
    dmask0 = np.zeros((128, 256), f32)
    half = 0 if p == 0 else 128
    mixfac_n = mix * 8.0 / (1.0 - mix)
    mixfac_0 = mix * counts[0] / (1.0 - mix)
    dmask[r, half + r] = mixfac_n
    dmask0[r, half + r] = mixfac_0

    recipc = np.ascontiguousarray(((1.0 - mix) / counts).T).astype(f32)

    wg = np.ascontiguousarray(np.asarray(w_gate, f32).reshape(DC, 128).T
                              ).astype(sdt_np)
    NP = 4 * 256 + NQT + 2 * DC + 4
    prm = np.empty((128, NP), f32)
    prm[:, 0:256] = qmask
    prm[:, 256:512] = smask
    prm[:, 512:768] = dmask
    prm[:, 768:1024] = dmask0
    prm[:, 1024:1024 + NQT] = recipc
    prm[:, 1024 + NQT:1024 + NQT + DC] = np.asarray(gain, f32).reshape(DC, 128).T
    prm[:, 1024 + NQT + DC:1024 + NQT + 2 * DC] = \
        np.asarray(bias, f32).reshape(DC, 128).T
    prm[:, 1024 + NQT + 2 * DC] = b_gate
    prm[:, 1024 + NQT + 2 * DC + 1] = scale
    prm[:, 1024 + NQT + 2 * DC + 2] = 1.0 if p == 0 else 0.0
    prm[:, 1024 + NQT + 2 * DC + 3] = 0.0 if p == 0 else 1.0
    return {"prm": prm, "wg": wg}


def _get_runner(T, reps=1, sdt=FP16, stage=5):
    """Build (or fetch) the compiled program + jitted 8-core dispatcher."""
    key = (T, reps, sdt, stage)
    if key in _runner_cache:
        return _runner_cache[key]

    import jax
    from jax.sharding import Mesh, PartitionSpec, NamedSharding
    from jax.experimental.shard_map import shard_map
    from concourse import bass2jax
    from concourse.bass2jax import _bass_exec_p, install_neuronx_cc_hook

    nc = build_program(T, reps=reps, sdt=sdt, stage=stage)
    install_neuronx_cc_hook()
    partition_name = nc.partition_id_tensor.name if nc.partition_id_tensor else None

    in_names, out_names, out_avals = [], [], []
    for alloc in nc.m.functions[0].allocations:
        if not isinstance(alloc, mybir.MemoryLocationSet):
            continue
        name = alloc.memorylocations[0].name
        if alloc.kind == "ExternalInput":
            if name != partition_name:
                in_names.append(name)
        elif alloc.kind == "ExternalOutput":
            shape = tuple(alloc.tensor_shape)
            dtype = mybir.dt.np(alloc.dtype)
            out_names.append(name)
            out_avals.append(jax.core.ShapedArray(shape, dtype))
    n_params = len(in_names)
    n_outs = len(out_names)
    all_in_names = list(in_names) + out_names
    if partition_name is not None:
        all_in_names.append(partition_name)

    def _body(*args):
        operands = list(args)
        if partition_name is not None:
            operands.append(bass2jax.partition_id_tensor())
        outs = _bass_exec_p.bind(
            *operands,
            out_avals=tuple(out_avals),
            in_names=tuple(all_in_names),
            out_names=tuple(out_names),
            lowering_input_output_aliases=(),
            sim_require_finite=True,
            sim_require_nnan=True,
            nc=nc,
        )
        return tuple(outs)

    devices = jax.devices()[:N_CORES]
    mesh = Mesh(np.asarray(devices), ("core",))
    sh = NamedSharding(mesh, PartitionSpec("core"))
    in_specs = (PartitionSpec("core"),) * (n_params + n_outs)
    out_specs = (PartitionSpec("core"),) * n_outs
    fn = jax.jit(shard_map(_body, mesh=mesh, in_specs=in_specs,
                           out_specs=out_specs, check_rep=False),
                 keep_unused=True)

    # device-resident zero buffers for the outputs (program writes every
    # element, so these are never read; reused across calls)
    zeros = []
    for av in out_avals:
        zfn = jax.jit(lambda shape=av.shape, dt=av.dtype:
                      jax.numpy.zeros((N_CORES * shape[0], *shape[1:]), dt),
                      out_shardings=sh)
        zeros.append(zfn())
    jax.block_until_ready(zeros)

    runner = dict(nc=nc, fn=fn, in_names=in_names, out_names=out_names,
                  zeros=zeros, mesh=mesh, sh=sh, devices=list(devices),
                  jax=jax)
    _runner_cache[key] = runner
    return runner


def run_cores(x, w_gate, b_gate, gain, bias, log_mix, log_scale,
              reps=1, sdt=FP16, stage=5, bench=False, return_raw=False):
    """Run the SPMD program over all 8 cores; returns (B,T,D) output."""
    x = np.asarray(x)
    B, T, _ = x.shape
    sdt_np = mybir.dt.np(sdt) if sdt == FP16 else np.float32
    mix = float(1.0 / (1.0 + np.exp(-np.float64(log_mix))))
    scale = float(np.logaddexp(0.0, np.float64(log_scale)) + 0.01)
    b_gate_f = float(np.asarray(b_gate, np.float64))

    rn = _get_runner(T, reps=reps, sdt=sdt, stage=stage)
    jax = rn["jax"]

    # XLA-on-CPU helpers: hardware fp16 casts + fused deinterleave, both
    # multithreaded (numpy's half casts are slow software loops)
    if "conv16" not in rn:
        cpu_dev = jax.devices("cpu")[0]
        rn["conv16"] = jax.jit(
            lambda a: a.astype(jax.numpy.float16), device=cpu_dev)
        rn["assemble"] = jax.jit(
            lambda a, B=B: a.reshape(B, 2, T // 256, 128, D)
            .transpose(0, 2, 1, 3, 4).reshape(B, T, D)
            .astype(jax.numpy.float32), device=cpu_dev)
    if x.dtype != sdt_np:
        x = np.asarray(rn["conv16"](x)) if sdt == FP16 else x.astype(sdt_np)

    small = [host_small(p, mix, scale, b_gate_f, w_gate, gain, bias, T,
                        sdt_np=sdt_np)
             for p in (0, 1)]

    # core 2b+p gets half p of x[b]; shard order over 8 cores is exactly
    # x.reshape(8*T/2, D), so one sharded device_put moves everything
    # (the program pair-AllGathers the full batch on device)
    assert B * T == N_CORES * (T // 2), "sharding specialized for B=4"
    H = T // 2
    gx = jax.device_put(x.reshape(N_CORES * H, D), rn["sh"])

    # params rarely change between calls: cache their device arrays by value
    import hashlib
    pkey = hashlib.md5(b"".join(small[p][n].tobytes()
                                for p in (0, 1)
                                for n in sorted(small[p]))).hexdigest()
    if rn.get("prm_key") != pkey:
        rn["prm_dev"] = {
            name: jax.device_put(
                np.concatenate([small[c & 1][name] for c in range(N_CORES)],
                               axis=0), rn["sh"])
            for name in rn["in_names"] if name != "x"}
        rn["prm_key"] = pkey
    dev_in = [gx if name == "x" else rn["prm_dev"][name]
              for name in rn["in_names"]]

    r = rn["fn"](*dev_in, *rn["zeros"])
    y_all = np.asarray(r[0]).reshape(N_CORES, T // 256, 128, D)
    if return_raw:
        return y_all

    return np.asarray(rn["assemble"](y_all))


def kernel(x, w_gate, b_gate, gain, bias, log_mix, log_scale, K):
    assert int(K) == 8, "kernel is specialized for K=8"
    return run_cores(x, w_gate, b_gate, gain, bias, log_mix, log_scale)



# revision 7
# speedup vs baseline: 15.4533x; 15.4533x over previous
"""Trainium2 Bass kernel for causal top-K (K=8) similarity message passing.

Math per batch b (reference):
  gate  = sigmoid(x @ w_gate + b_gate)                      (B,T)
  S     = x @ x^T, causal-masked to NEG=-1e30               (B,T,T)
  top-8 per row -> selected neighbour set, count=min(i+1,8)
  msg   = mean of selected x rows
  blend = mix*x + (1-mix)*msg
  out   = gate * gelu(blend*gain + bias) * (softplus(log_scale)+0.01)

Sharding: 8 cores = 4 batches x 2 query-parity shards. Core c handles
batch b=c>>1, parity p=c&1, processing query tiles g = 2t+p (t=0..T/256-1)
of 128 rows each. One uniform program for all cores; all parity
dependence is carried in per-core input data (masks / select scalars).

Per-core input is ONLY half of x[b] in fp16 (4MB, zero-copy upload;
the pair AllGathers the full batch on device) plus one packed ~0.5MB
param/mask tensor. Everything else is computed on device:

  xfull   = pair AllGather of the two contiguous x[b] halves
  xT      = xfull^T built once via PE transposes (fp16)
  per query tile t (Lc = 2t+2 key chunks of 128):
    xqT    = c1*xT[:, 2t*128 win] + c2*xT[:, (2t+1)*128 win]
             (both parity candidates live in xT; c1/c2 are input data)
    scores = xqT^T @ xT (PE, fp16 in / f32 acc) -> SBUF + causal masks
    v8     = max8(scores), tau = v8[:,7]       (DVE top-8 instruction)
    sel    = scores >= tau (0/1), fixed up for tile 0, diagonal gets
             mix*count/(1-mix) added
    pm     = sum_c sel_chunk^T-transposed @ x_chunk        (PSUM f32)
    msg    = pm * (1-mix)/count          (per-row scale = blend[q, d])
    y      = gate * transpose(Gelu(transpose-slab(msg)*gainT + biasT))
             (gain/bias/gelu applied in feature-on-partition layout)

Output y is fp16 (halves the download); the host deinterleaves and
upcasts via one fused XLA-on-CPU op.
"""

import os
import sys

for _p in ("/opt/trn_rl_repo", os.path.expanduser("~/.axon_site/_ro/trn_rl_repo")):
    if os.path.isdir(_p) and _p not in sys.path:
        sys.path.insert(0, _p)
        break

import numpy as np

import concourse.bacc as bacc
import concourse.mybir as mybir
from concourse import masks
from concourse.tile import TileContext

F32 = mybir.dt.float32
F32R = mybir.dt.float32r
FP16 = mybir.dt.float16
I32 = mybir.dt.int32
AF = mybir.ActivationFunctionType
ALU = mybir.AluOpType
NEG = np.float32(-1e30)

D = 1024
DC = 8  # D // 128
N_CORES = 8

_prog_cache = {}
_runner_cache = {}


def build_program(T, reps=1, sdt=FP16, stage=5):
    """Build + compile the uniform per-core program for sequence length T."""
    key = (T, reps, sdt, stage)
    if key in _prog_cache:
        return _prog_cache[key]

    ODT = FP16 if sdt == FP16 else F32   # output dtype
    PDT = sdt                            # PSUM dtype for sdt transposes

    NQT = T // 256  # query tiles per core
    nc = bacc.Bacc(trn_type="TRN2", target_bir_lowering=False, debug=False,
                   num_devices=N_CORES, dynamic_dma_scratch_size=512)

    NP = 4 * 256 + NQT + 2 * DC + 4      # packed f32 param columns
    x_in = nc.dram_tensor("x", [T, D], sdt, kind="ExternalInput").ap()
    prm_in = nc.dram_tensor("prm", [128, NP], F32, kind="ExternalInput").ap()
    WDT = sdt if sdt == FP16 else F32    # gate weight dtype
    wg_in = nc.dram_tensor("wg", [128, DC], WDT, kind="ExternalInput").ap()
    y_out = nc.dram_tensor("y", [NQT, 128, D], ODT, kind="ExternalOutput").ap()

    from contextlib import ExitStack

    with TileContext(nc) as tc, ExitStack() as ctx:
        cpool = ctx.enter_context(tc.tile_pool(name="consts", bufs=1))
        xTp = ctx.enter_context(tc.tile_pool(name="xTp", bufs=1))
        xallp = ctx.enter_context(tc.tile_pool(name="xallp", bufs=1))
        Sp = ctx.enter_context(tc.tile_pool(name="Sp", bufs=2))
        xqp = ctx.enter_context(tc.tile_pool(name="xqp", bufs=2))
        stp = ctx.enter_context(tc.tile_pool(name="stp", bufs=3))
        msgp = ctx.enter_context(tc.tile_pool(name="msgp", bufs=3))
        smallp = ctx.enter_context(tc.tile_pool(name="smallp", bufs=2))
        ps_s = ctx.enter_context(tc.tile_pool(name="ps_s", bufs=2, space="PSUM"))
        ps_t = ctx.enter_context(tc.tile_pool(name="ps_t", bufs=3, space="PSUM"))
        ps_m = ctx.enter_context(tc.tile_pool(name="ps_m", bufs=1, space="PSUM"))
        ps_g = ctx.enter_context(tc.tile_pool(name="ps_g", bufs=1, space="PSUM"))

        prm = cpool.tile([128, NP], F32)
        nc.sync.dma_start(out=prm[:], in_=prm_in[:])
        qmask = prm[:, 0:256]
        smask = prm[:, 256:512]
        dmask = prm[:, 512:768]
        dmask0 = prm[:, 768:1024]
        recip = prm[:, 1024:1024 + NQT]
        gainT = prm[:, 1024 + NQT:1024 + NQT + DC]
        biasT = prm[:, 1024 + NQT + DC:1024 + NQT + 2 * DC]
        sc = prm[:, 1024 + NQT + 2 * DC:1024 + NQT + 2 * DC + 4]
        wg = cpool.tile([128, DC], WDT)
        nc.sync.dma_start(out=wg[:], in_=wg_in[:])
        ident32 = cpool.tile([128, 128], F32)
        masks.make_identity(nc, ident32[:])
        identR = cpool.tile([128, 128], sdt)
        nc.scalar.copy(identR[:], ident32[:])

        for _rep in range(reps):
            # ---- each core holds the FULL batch x[b] (uploaded host-side;
            # no on-device collective). Load all key chunks into SBUF once;
            # build xT[dp, dc, k] = x[k, dc*128+dp] via PE transposes ----
            xT = xTp.tile([128, DC, T], sdt)
            xall = xallp.tile([128, T // 128, D], sdt)
            for c in range(T // 128):
                nc.sync.dma_start(out=xall[:, c],
                                  in_=x_in[c * 128:(c + 1) * 128, :])
                for dc in range(DC):
                    pt = ps_t.tile([128, 128], PDT, tag="pt", name="ptr")
                    nc.tensor.transpose(pt[:],
                                        xall[:, c, dc * 128:(dc + 1) * 128],
                                        identR[:])
                    nc.scalar.copy(xT[:, dc, c * 128:(c + 1) * 128], pt[:])

            if stage <= 1:
                dbg = msgp.tile([128, D], ODT, name="dbg")
                nc.vector.tensor_copy(dbg[:], xT[:, 0, 0:D])
                nc.sync.dma_start(out=y_out[0], in_=dbg[:])
                continue

            for t in range(NQT):
                Lc = 2 * t + 2
                Lk = Lc * 128

                # ---- extract this core's query tile (parity is data) ----
                # both parity candidates already live transposed inside xT:
                #   xqT = c1*xT[:, :, 2t*128 window] + c2*xT[:, :, (2t+1)*128]
                #       = (w0 - w1)*c1 + w1          (c1, c2=1-c1 in {0,1})
                xqT = xqp.tile([128, DC, 128], sdt, tag="xqT", name="xqT")
                w0 = xT[:, :, 2 * t * 128:(2 * t) * 128 + 128]
                w1 = xT[:, :, (2 * t + 1) * 128:(2 * t + 1) * 128 + 128]
                nc.vector.tensor_sub(xqT[:], w0, w1)
                nc.vector.tensor_scalar(xqT[:], xqT[:], sc[:, 2:3], None,
                                        op0=ALU.mult)
                nc.vector.tensor_add(xqT[:], xqT[:], w1)

                # ---- scores ----
                S = Sp.tile([128, T], F32)
                nblk = (Lk + 511) // 512
                for blk in range(nblk):
                    w = min(512, Lk - blk * 512)
                    ps = ps_s.tile([128, 512], F32)
                    for dc in range(DC):
                        nc.tensor.matmul(ps[:, :w], xqT[:, dc],
                                         xT[:, dc, blk * 512:blk * 512 + w],
                                         start=(dc == 0), stop=(dc == DC - 1))
                    lo = blk * 512
                    plain_w = w if blk < nblk - 1 else w - 256
                    if plain_w > 0:
                        nc.scalar.copy(S[:, lo:lo + plain_w], ps[:, :plain_w])
                    if blk == nblk - 1:
                        nc.vector.tensor_add(S[:, Lk - 256:Lk],
                                             ps[:, w - 256:w], qmask)

                if stage <= 2:
                    dbg2 = msgp.tile([128, D], ODT, name="dbg2")
                    nc.vector.tensor_copy(dbg2[:], S[:, 0:D])
                    nc.sync.dma_start(out=y_out[t], in_=dbg2[:])
                    continue

                # ---- top-8 threshold -> selection weights in-place ----
                v8 = smallp.tile([128, 8], F32, tag="v8", name="v8")
                nc.vector.max(out=v8[:], in_=S[:, :Lk])
                nc.vector.tensor_scalar(S[:, :Lk], S[:, :Lk], v8[:, 7:8], None,
                                        op0=ALU.is_ge)
                if t == 0:
                    nc.vector.tensor_mul(S[:, :256], S[:, :256], smask)
                dm = dmask0 if t == 0 else dmask
                nc.vector.tensor_add(S[:, Lk - 256:Lk], S[:, Lk - 256:Lk], dm)

                if stage <= 3:
                    dbg3 = msgp.tile([128, D], ODT, name="dbg3")
                    nc.vector.tensor_copy(dbg3[:], S[:, 0:D])
                    nc.sync.dma_start(out=y_out[t], in_=dbg3[:])
                    continue

                # ---- gate ----
                pg = ps_g.tile([128, 1], F32)
                for dc in range(DC):
                    lq = (xqT[:, dc] if sdt == FP16
                          else xqT[:, dc].bitcast(F32))
                    nc.tensor.matmul(pg[:], lq, wg[:, dc:dc + 1],
                                     start=(dc == 0), stop=(dc == DC - 1))
                gate = smallp.tile([128, 1], F32, tag="gate", name="gate")
                nc.scalar.activation(gate[:], pg[:], AF.Sigmoid,
                                     bias=sc[:, 0:1], scale=1.0)
                nc.vector.tensor_mul(gate[:], gate[:], sc[:, 1:2])

                # ---- aggregation: pm[q, d] = sum_j selw[q, j] * x[j, d] ----
                pm = ps_m.tile([128, D], F32)
                for c in range(Lc):
                    pt = ps_t.tile([128, 128], F32, tag="pt", name="pts")
                    nc.tensor.transpose(pt[:], S[:, c * 128:(c + 1) * 128],
                                        ident32[:])
                    sT = stp.tile([128, 128], sdt)
                    nc.scalar.copy(sT[:], pt[:])
                    for h in (0, 1):
                        nc.tensor.matmul(pm[:, h * 512:(h + 1) * 512], sT[:],
                                         xall[:, c, h * 512:(h + 1) * 512],
                                         start=(c == 0), stop=(c == Lc - 1))

                # msg = blend[q, d] = pm * (1-mix)/count   (per-row scale)
                msg = msgp.tile([128, D], F32)
                nc.scalar.activation(msg[:], pm[:], AF.Copy,
                                     scale=recip[:, t:t + 1])

                if stage <= 4:
                    dbg4 = msgp.tile([128, D], ODT, name="dbg4")
                    nc.vector.tensor_copy(dbg4[:], msg[:])
                    nc.sync.dma_start(out=y_out[t], in_=dbg4[:])
                    continue

                # ---- tail: y = gate * Gelu(blend^T*gainT + biasT)^T ----
                y = msgp.tile([128, D], ODT, tag="y", name="y")
                for dc in range(DC):
                    pt1 = ps_t.tile([128, 128], F32, tag="pt", name="pt1")
                    nc.tensor.transpose(pt1[:], msg[:, dc * 128:(dc + 1) * 128],
                                        ident32[:])
                    gels = stp.tile([128, 128], F32, tag="gels", name="gels")
                    nc.scalar.activation(gels[:], pt1[:], AF.Gelu,
                                         bias=biasT[:, dc:dc + 1],
                                         scale=gainT[:, dc:dc + 1])
                    pt2 = ps_t.tile([128, 128], F32, tag="pt", name="pt2")
                    nc.tensor.transpose(pt2[:], gels[:], ident32[:])
                    nc.scalar.activation(y[:, dc * 128:(dc + 1) * 128], pt2[:],
                                         AF.Copy, scale=gate[:, 0:1])
                nc.sync.dma_start(out=y_out[t], in_=y[:])

    nc.compile()
    _prog_cache[key] = nc
    return nc


def host_small(p, mix, scale, b_gate, w_gate, gain, bias, T, sdt_np=np.float16):
    """Small per-core input arrays for parity p (everything except x)."""
    NQT = T // 256
    f32 = np.float32

    r = np.arange(128)
    tri_add = np.where(r[None, :] <= r[:, None], f32(0), NEG).astype(f32)
    tri01 = (r[None, :] <= r[:, None]).astype(f32)
    qmask = np.zeros((128, 256), f32)
    smask = np.zeros((128, 256), f32)
    if p == 0:
        qmask[:, :128] = tri_add
        qmask[:, 128:] = NEG
        smask[:, :128] = tri01
    else:
        qmask[:, 128:] = tri_add
        smask[:, :128] = 1.0
        smask[:, 128:] = tri01

    # counts: count(t, q) = min((2t+p)*128 + q + 1, 8)
    g_row = (2 * np.arange(NQT)[:, None] + p) * 128 + r[None, :]  # (NQT,128)
    counts = np.minimum(g_row + 1, 8).astype(f32)

    dmask = np.zeros((128, 256), f32)
    dmask0 = np.zeros((128, 256), f32)
    half = 0 if p == 0 else 128
    mixfac_n = mix * 8.0 / (1.0 - mix)
    mixfac_0 = mix * counts[0] / (1.0 - mix)
    dmask[r, half + r] = mixfac_n
    dmask0[r, half + r] = mixfac_0

    recipc = np.ascontiguousarray(((1.0 - mix) / counts).T).astype(f32)

    wg = np.ascontiguousarray(np.asarray(w_gate, f32).reshape(DC, 128).T
                              ).astype(sdt_np)
    NP = 4 * 256 + NQT + 2 * DC + 4
    prm = np.empty((128, NP), f32)
    prm[:, 0:256] = qmask
    prm[:, 256:512] = smask
    prm[:, 512:768] = dmask
    prm[:, 768:1024] = dmask0
    prm[:, 1024:1024 + NQT] = recipc
    prm[:, 1024 + NQT:1024 + NQT + DC] = np.asarray(gain, f32).reshape(DC, 128).T
    prm[:, 1024 + NQT + DC:1024 + NQT + 2 * DC] = \
        np.asarray(bias, f32).reshape(DC, 128).T
    prm[:, 1024 + NQT + 2 * DC] = b_gate
    prm[:, 1024 + NQT + 2 * DC + 1] = scale
    prm[:, 1024 + NQT + 2 * DC + 2] = 1.0 if p == 0 else 0.0
    prm[:, 1024 + NQT + 2 * DC + 3] = 0.0 if p == 0 else 1.0
    return {"prm": prm, "wg": wg}


def _get_runner(T, reps=1, sdt=FP16, stage=5):
    """Build (or fetch) the compiled program + jitted 8-core dispatcher."""
    key = (T, reps, sdt, stage)
    if key in _runner_cache:
        return _runner_cache[key]

    import jax
    from jax.sharding import Mesh, PartitionSpec, NamedSharding
    from jax.experimental.shard_map import shard_map
    from concourse import bass2jax
    from concourse.bass2jax import _bass_exec_p, install_neuronx_cc_hook

    nc = build_program(T, reps=reps, sdt=sdt, stage=stage)
    install_neuronx_cc_hook()
    partition_name = nc.partition_id_tensor.name if nc.partition_id_tensor else None

    in_names, out_names, out_avals = [], [], []
    for alloc in nc.m.functions[0].allocations:
        if not isinstance(alloc, mybir.MemoryLocationSet):
            continue
        name = alloc.memorylocations[0].name
        if alloc.kind == "ExternalInput":
            if name != partition_name:
                in_names.append(name)
        elif alloc.kind == "ExternalOutput":
            shape = tuple(alloc.tensor_shape)
            dtype = mybir.dt.np(alloc.dtype)
            out_names.append(name)
            out_avals.append(jax.core.ShapedArray(shape, dtype))
    n_params = len(in_names)
    n_outs = len(out_names)
    all_in_names = list(in_names) + out_names
    if partition_name is not None:
        all_in_names.append(partition_name)

    def _body(*args):
        operands = list(args)
        if partition_name is not None:
            operands.append(bass2jax.partition_id_tensor())
        outs = _bass_exec_p.bind(
            *operands,
            out_avals=tuple(out_avals),
            in_names=tuple(all_in_names),
            out_names=tuple(out_names),
            lowering_input_output_aliases=(),
            sim_require_finite=True,
            sim_require_nnan=True,
            nc=nc,
        )
        return tuple(outs)

    devices = jax.devices()[:N_CORES]
    mesh = Mesh(np.asarray(devices), ("core",))
    sh = NamedSharding(mesh, PartitionSpec("core"))
    in_specs = (PartitionSpec("core"),) * (n_params + n_outs)
    out_specs = (PartitionSpec("core"),) * n_outs
    fn = jax.jit(shard_map(_body, mesh=mesh, in_specs=in_specs,
                           out_specs=out_specs, check_rep=False),
                 keep_unused=True)

    # device-resident zero buffers for the outputs (program writes every
    # element, so these are never read; reused across calls)
    zeros = []
    for av in out_avals:
        zfn = jax.jit(lambda shape=av.shape, dt=av.dtype:
                      jax.numpy.zeros((N_CORES * shape[0], *shape[1:]), dt),
                      out_shardings=sh)
        zeros.append(zfn())
    jax.block_until_ready(zeros)

    runner = dict(nc=nc, fn=fn, in_names=in_names, out_names=out_names,
                  zeros=zeros, mesh=mesh, sh=sh, devices=list(devices),
                  jax=jax)
    _runner_cache[key] = runner
    return runner


def run_cores(x, w_gate, b_gate, gain, bias, log_mix, log_scale,
              reps=1, sdt=FP16, stage=5, bench=False, return_raw=False):
    """Run the SPMD program over all 8 cores; returns (B,T,D) output."""
    x = np.asarray(x)
    B, T, _ = x.shape
    sdt_np = mybir.dt.np(sdt) if sdt == FP16 else np.float32
    mix = float(1.0 / (1.0 + np.exp(-np.float64(log_mix))))
    scale = float(np.logaddexp(0.0, np.float64(log_scale)) + 0.01)
    b_gate_f = float(np.asarray(b_gate, np.float64))

    rn = _get_runner(T, reps=reps, sdt=sdt, stage=stage)
    jax = rn["jax"]

    # XLA-on-CPU helpers: hardware fp16 casts + fused deinterleave, both
    # multithreaded (numpy's half casts are slow software loops)
    if "conv16" not in rn:
        cpu_dev = jax.devices("cpu")[0]
        rn["conv16"] = jax.jit(
            lambda a: a.astype(jax.numpy.float16)[
                jax.numpy.repeat(jax.numpy.arange(B), 2)]
            .reshape(N_CORES * T, D), device=cpu_dev)
        rn["assemble"] = jax.jit(
            lambda a, B=B: a.reshape(B, 2, T // 256, 128, D)
            .transpose(0, 2, 1, 3, 4).reshape(B, T, D)
            .astype(jax.numpy.float32), device=cpu_dev)
    # core 2b+p gets the FULL batch x[b] (duplicated across the pair)
    x = np.asarray(rn["conv16"](x))

    small = [host_small(p, mix, scale, b_gate_f, w_gate, gain, bias, T,
                        sdt_np=sdt_np)
             for p in (0, 1)]

    assert 2 * B == N_CORES, "sharding specialized for B=4"
    gx = jax.device_put(x, rn["sh"])

    # params rarely change between calls: cache their device arrays by value
    import hashlib
    pkey = hashlib.md5(b"".join(small[p][n].tobytes()
                                for p in (0, 1)
                                for n in sorted(small[p]))).hexdigest()
    if rn.get("prm_key") != pkey:
        rn["prm_dev"] = {
            name: jax.device_put(
                np.concatenate([small[c & 1][name] for c in range(N_CORES)],
                               axis=0), rn["sh"])
            for name in rn["in_names"] if name != "x"}
        rn["prm_key"] = pkey
    dev_in = [gx if name == "x" else rn["prm_dev"][name]
              for name in rn["in_names"]]

    r = rn["fn"](*dev_in, *rn["zeros"])
    y_all = np.asarray(r[0]).reshape(N_CORES, T // 256, 128, D)
    if return_raw:
        return y_all

    return np.asarray(rn["assemble"](y_all))


def kernel(x, w_gate, b_gate, gain, bias, log_mix, log_scale, K):
    assert int(K) == 8, "kernel is specialized for K=8"
    return run_cores(x, w_gate, b_gate, gain, bias, log_mix, log_scale)

